# revision 8
# baseline (speedup 1.0000x reference)
"""Trainium2 Bass kernel for nn_AttentionBlock (GroupNorm + single-head
self-attention over H*W tokens + proj + residual).

Sharding: data-parallel over batch N=8 -> one image per NeuronCore (8 cores).
Per-core layout: x as [C=512, T=4096] f32.

Pipeline per core:
  1. GroupNorm(32 groups): channel stats split across DVE (bn_stats) and
     ScalarE (Copy/Square activations with accum_out); group-combine across
     partitions via tiny f32 selector matmuls on PE; fused (a*x+b) normalize
     on ScalarE -> h bf16 [C, T].
  2. QKV: bf16 matmuls. Q, K in [C, T] layout; V computed directly
     transposed as [T, C].
  3. Attention per q-block of 512 queries: S^T[k, q] tiles from PE,
     exp on ScalarE (scale=1/sqrt(C) folded, no max subtraction -- scores
     are O(6) for this problem), output lands as bf16 P^T directly usable
     as PV stationary operand. Row sums via all-ones [128,128] matmul so the
     sum lands broadcast across partitions; reciprocal on DVE PSUM->SBUF.
     o = V^T.T @ P^T accumulated in PSUM as [C, q]; normalized by VectorE
     multiply with the broadcast reciprocal.
  4. proj matmul + f32 residual add into [128, T/2] staging, 2 DMAs per
     c-tile out.
"""

import numpy as np
import ml_dtypes

import concourse.bass as bass
import concourse.mybir as mybir
import concourse.tile as tile
from concourse import bacc
from concourse.bass_utils import run_bass_kernel_spmd

F32 = mybir.dt.float32
BF16 = mybir.dt.bfloat16
AF = mybir.ActivationFunctionType

N, C, H, W = 8, 512, 64, 64
T = H * W                 # 4096
G = 32                    # groups
GS = C // G               # 16 channels per group
CT = C // 128             # 4 c-tiles of 128
TT = T // 128             # 32 t-tiles of 128
TQ = T // 512             # 8 t-chunks of 512
GPT = 128 // GS           # 8 groups per c-tile
EPS = 1e-5
SCALE = float(C) ** -0.5

_NC_CACHE = {}


def _build(has_vb: bool, has_pb: bool):
    key = (has_vb, has_pb)
    if key in _NC_CACHE:
        return _NC_CACHE[key]

    XH = T // 2               # x half-tile width
    BC = min(512, XH)         # bn_stats chunk width
    NBH = XH // BC            # bn chunks per half

    nc = bacc.Bacc("TRN2", target_bir_lowering=False, debug=False, num_devices=8)

    xd = nc.dram_tensor("x", [C, T], F32, kind="ExternalInput").ap()
    wq = nc.dram_tensor("wq", [C, C], BF16, kind="ExternalInput").ap()
    wk = nc.dram_tensor("wk", [C, C], BF16, kind="ExternalInput").ap()
    wv = nc.dram_tensor("wv", [C, C], BF16, kind="ExternalInput").ap()
    wp = nc.dram_tensor("wp", [C, C], BF16, kind="ExternalInput").ap()
    gnw = nc.dram_tensor("gnw", [128, CT], F32, kind="ExternalInput").ap()
    gnb = nc.dram_tensor("gnb", [128, CT], F32, kind="ExternalInput").ap()
    qb = nc.dram_tensor("qb", [128, CT], F32, kind="ExternalInput").ap()
    kb = nc.dram_tensor("kb", [128, CT], F32, kind="ExternalInput").ap()
    vb = nc.dram_tensor("vb", [1, C], F32, kind="ExternalInput").ap()
    pb = nc.dram_tensor("pb", [128, CT], F32, kind="ExternalInput").ap()
    gsel = nc.dram_tensor("gsel", [128, GPT], F32, kind="ExternalInput").ap()
    gselT = nc.dram_tensor("gselT", [GPT, 128], F32, kind="ExternalInput").ap()
    onesbf = nc.dram_tensor("onesbf", [128, 128], BF16, kind="ExternalInput").ap()
    ones1r = nc.dram_tensor("ones1r", [1, 128], F32, kind="ExternalInput").ap()
    outd = nc.dram_tensor("out", [C, T], F32, kind="ExternalOutput").ap()

    with tile.TileContext(nc) as tc:
        with (
            tc.tile_pool(name="consts", bufs=1) as consts,
            tc.tile_pool(name="weights", bufs=1) as wpool,
            tc.tile_pool(name="small", bufs=2) as small,
            tc.tile_pool(name="rows", bufs=2) as rows,
            tc.tile_pool(name="hopool", bufs=1) as hopool,
            tc.tile_pool(name="mm_psum", bufs=3, space="PSUM") as mm_psum,
        ):
            # ---- GroupNorm (x resident only in this scope) ----
            # x DMAs issue first on the sync (HWDGE) path, half-tiles on
            # separate queues; consts+weights go via gpsimd (SWDGE) so their
            # issue cost doesn't delay x.
            h_sb = []
            with (
                tc.tile_pool(name="xpool", bufs=1) as xpool,
                tc.tile_pool(name="gn_psum", bufs=1, space="PSUM") as gn_psum,
            ):
                x_sb = []  # [ct][half]
                for ct in range(CT):
                    pair = []
                    for hf in range(2):
                        xt = xpool.tile([128, XH], F32, tag=f"x{ct}{hf}", name=f"x{ct}{hf}")
                        nc.sync.dma_start(
                            out=xt,
                            in_=xd[ct * 128:(ct + 1) * 128, hf * XH:(hf + 1) * XH])
                        pair.append(xt)
                    x_sb.append(pair)

                # ---- constants / weights (SWDGE path) ----
                gsel_sb = consts.tile([128, GPT], F32, tag="gsel")
                nc.gpsimd.dma_start(out=gsel_sb, in_=gsel)
                gselT_sb = consts.tile([GPT, 128], F32, tag="gselT")
                nc.gpsimd.dma_start(out=gselT_sb, in_=gselT)
                gnw_sb = consts.tile([128, CT], F32, tag="gnw")
                nc.gpsimd.dma_start(out=gnw_sb, in_=gnw)
                gnb_sb = consts.tile([128, CT], F32, tag="gnb")
                nc.gpsimd.dma_start(out=gnb_sb, in_=gnb)
                qb_sb = consts.tile([128, CT], F32, tag="qb")
                nc.gpsimd.dma_start(out=qb_sb, in_=qb)
                kb_sb = consts.tile([128, CT], F32, tag="kb")
                nc.gpsimd.dma_start(out=kb_sb, in_=kb)
                ones_sb = consts.tile([128, 128], BF16, tag="onesbf")
                nc.gpsimd.dma_start(out=ones_sb, in_=onesbf)
                ones1r_sb = consts.tile([1, 128], F32, tag="ones1r")
                nc.gpsimd.dma_start(out=ones1r_sb, in_=ones1r)
                eps_sb = consts.tile([GPT, 1], F32, tag="eps")
                nc.vector.memset(eps_sb, EPS)
                if has_pb:
                    pb_sb = consts.tile([128, CT], F32, tag="pb")
                    nc.gpsimd.dma_start(out=pb_sb, in_=pb)
                if has_vb:
                    vb_sb = consts.tile([1, C], F32, tag="vbrow")
                    nc.gpsimd.dma_start(out=vb_sb, in_=vb)

                wq_sb, wk_sb, wv_sb, wp_sb = [], [], [], []
                for name, src, lst in (
                    ("wq", wq, wq_sb), ("wk", wk, wk_sb),
                    ("wv", wv, wv_sb), ("wp", wp, wp_sb),
                ):
                    for cc in range(CT):
                        wt = wpool.tile([128, C], BF16, tag=f"{name}{cc}", name=f"{name}{cc}")
                        nc.gpsimd.dma_start(out=wt, in_=src[cc * 128:(cc + 1) * 128, :])
                        lst.append(wt)

                # ---- channel stats: tiles 0-1 on DVE, tiles 2-3 on ScalarE ----
                stats8 = small.tile([128, 2 * CT], F32, tag="stats8")
                for ct in (0, 1):
                    bn6 = small.tile([128, 2 * NBH, 6], F32, tag="bn6")
                    for hf in range(2):
                        for j in range(NBH):
                            nc.vector.bn_stats(
                                out=bn6[:, hf * NBH + j, :],
                                in_=x_sb[ct][hf][:, j * BC:(j + 1) * BC])
                    mv = small.tile([128, 2], F32, tag="mv")
                    nc.vector.bn_aggr(out=mv, in_=bn6)
                    nc.vector.tensor_copy(stats8[:, ct:ct + 1], mv[:, 0:1])
                    msq = small.tile([128, 1], F32, tag="msq")
                    nc.vector.tensor_mul(msq, mv[:, 0:1], mv[:, 0:1])
                    nc.vector.tensor_add(stats8[:, CT + ct:CT + ct + 1], mv[:, 1:2], msq)
                sqs = float(T) ** -0.5
                for ct in (2, 3):
                    acc = small.tile([128, 4], F32, tag="acc", name=f"acc{ct}")
                    for hf in range(2):
                        scr = small.tile([128, XH], BF16, tag="gnscr", name=f"scr{ct}{hf}")
                        nc.scalar.activation(scr, x_sb[ct][hf], AF.Copy,
                                             scale=1.0 / T,
                                             accum_out=acc[:, hf:hf + 1])
                        scr2 = small.tile([128, XH], BF16, tag="gnscr2", name=f"scr2{ct}{hf}")
                        nc.scalar.activation(scr2, x_sb[ct][hf], AF.Square,
                                             scale=sqs,
                                             accum_out=acc[:, 2 + hf:3 + hf])
                    nc.vector.tensor_add(stats8[:, ct:ct + 1], acc[:, 0:1], acc[:, 1:2])
                    nc.vector.tensor_add(stats8[:, CT + ct:CT + ct + 1],
                                         acc[:, 2:3], acc[:, 3:4])

                g88_ps = gn_psum.tile([GPT, 2 * CT], F32, tag="gn_ps")
                nc.tensor.matmul(g88_ps, gsel_sb, stats8)
                g88 = small.tile([GPT, 2 * CT], F32, tag="g88")
                nc.scalar.copy(g88, g88_ps)
                t88 = small.tile([GPT, CT], F32, tag="t88")
                nc.vector.tensor_mul(t88, g88[:, 0:CT], g88[:, 0:CT])
                v88 = small.tile([GPT, CT], F32, tag="v88")
                nc.vector.tensor_sub(v88, g88[:, CT:2 * CT], t88)
                s88 = small.tile([GPT, CT], F32, tag="s88")
                nc.scalar.activation(s88, v88, AF.Sqrt, bias=eps_sb, scale=1.0)
                r88 = small.tile([GPT, CT], F32, tag="r88")
                nc.vector.reciprocal(r88, s88)
                br8 = small.tile([GPT, 2 * CT], F32, tag="br8")
                nc.vector.tensor_copy(br8[:, 0:CT], r88)
                nc.vector.tensor_copy(br8[:, CT:2 * CT], g88[:, 0:CT])
                bc_ps = gn_psum.tile([128, 2 * CT], F32, tag="gn_ps2")
                nc.tensor.matmul(bc_ps, gselT_sb, br8)
                bc = small.tile([128, 2 * CT], F32, tag="bc8")
                nc.scalar.copy(bc, bc_ps)
                a4 = small.tile([128, CT], F32, tag="a4")
                nc.vector.tensor_mul(a4, bc[:, 0:CT], gnw_sb)
                t4 = small.tile([128, CT], F32, tag="t4")
                nc.vector.tensor_mul(t4, a4, bc[:, CT:2 * CT])
                b4 = small.tile([128, CT], F32, tag="b4")
                nc.vector.tensor_sub(b4, gnb_sb, t4)

                for ct in range(CT):
                    ht = hopool.tile([128, T], BF16, tag=f"ho{ct}", name=f"h{ct}")
                    for hf in range(2):
                        nc.scalar.activation(ht[:, hf * XH:(hf + 1) * XH],
                                             x_sb[ct][hf], AF.Identity,
                                             scale=a4[:, ct:ct + 1],
                                             bias=b4[:, ct:ct + 1])
                    h_sb.append(ht)

            # ---- QKV + attention ----
            with (
                tc.tile_pool(name="qkpool", bufs=1) as qkpool,
                tc.tile_pool(name="vtiles", bufs=1) as vpool,
                tc.tile_pool(name="pt", bufs=TT + 2) as ptpool,
                tc.tile_pool(name="o_psum", bufs=2, space="PSUM") as o_psum,
                tc.tile_pool(name="rs_psum", bufs=2, space="PSUM") as rs_psum,
            ):
                if has_vb:
                    vbb_ps = rs_psum.tile([128, C], F32, tag="rs", name="vbb_ps")
                    nc.tensor.matmul(vbb_ps, ones1r_sb, vb_sb)
                    vb_bcast = consts.tile([128, C], F32, tag="vbb")
                    nc.scalar.copy(vb_bcast, vbb_ps)

                q_sb, k_sb = [], []
                for lst, w_t, b_t, nm in ((q_sb, wq_sb, qb_sb, "q"),
                                          (k_sb, wk_sb, kb_sb, "k")):
                    for mt in range(CT):
                        dst = qkpool.tile([128, T], BF16, tag=f"{nm}{mt}", name=f"{nm}{mt}")
                        lst.append(dst)
                        for tq in range(TQ):
                            ps = mm_psum.tile([128, 512], F32, tag="mm")
                            for cc in range(CT):
                                nc.tensor.matmul(
                                    ps,
                                    w_t[cc][:, mt * 128:(mt + 1) * 128],
                                    h_sb[cc][:, tq * 512:(tq + 1) * 512],
                                    start=(cc == 0), stop=(cc == CT - 1),
                                )
                            nc.scalar.activation(dst[:, tq * 512:(tq + 1) * 512], ps,
                                                 AF.Identity, scale=1.0,
                                                 bias=b_t[:, mt:mt + 1])

                v_sb = []
                for tt in range(TT):
                    vt = vpool.tile([128, C], BF16, tag=f"v{tt}", name=f"v{tt}")
                    ps = mm_psum.tile([128, 512], F32, tag="mm")
                    for cc in range(CT):
                        nc.tensor.matmul(
                            ps,
                            h_sb[cc][:, tt * 128:(tt + 1) * 128],
                            wv_sb[cc],
                            start=(cc == 0), stop=(cc == CT - 1),
                        )
                    if has_vb:
                        nc.vector.tensor_add(vt, ps, vb_bcast)
                    else:
                        nc.scalar.copy(vt, ps)
                    v_sb.append(vt)

                # ---- attention ----
                o_sb = [hopool.tile([128, T], BF16, tag=f"ho{ct}", name=f"o{ct}")
                        for ct in range(CT)]
                for qbk in range(TQ):
                    qs = slice(qbk * 512, (qbk + 1) * 512)
                    rs_ps = rs_psum.tile([128, 512], F32, tag="rs", name=f"rs{qbk}")
                    pt_tiles = []
                    for kt in range(TT):
                        st_ps = mm_psum.tile([128, 512], F32, tag="mm")
                        for cc in range(CT):
                            nc.tensor.matmul(
                                st_ps,
                                k_sb[cc][:, kt * 128:(kt + 1) * 128],
                                q_sb[cc][:, qs],
                                start=(cc == 0), stop=(cc == CT - 1),
                            )
                        pt = ptpool.tile([128, 512], BF16, tag="pt", name=f"pt{qbk}_{kt}")
                        nc.scalar.activation(pt, st_ps, AF.Exp, scale=SCALE)
                        pt_tiles.append(pt)
                        # all-ones stationary: every output partition gets the
                        # column sum -> row-sum lands pre-broadcast in PSUM
                        nc.tensor.matmul(rs_ps, ones_sb, pt,
                                         start=(kt == 0), stop=(kt == TT - 1))

                    bc_sb = rows.tile([128, 512], F32, tag="bcsb")
                    nc.vector.reciprocal(bc_sb, rs_ps)

                    for ct in range(CT):
                        ops = o_psum.tile([128, 512], F32, tag="o")
                        for kt in range(TT):
                            nc.tensor.matmul(
                                ops,
                                v_sb[kt][:, ct * 128:(ct + 1) * 128],
                                pt_tiles[kt],
                                start=(kt == 0), stop=(kt == TT - 1),
                            )
                        nc.vector.tensor_mul(o_sb[ct][:, qs], ops, bc_sb)

            # ---- proj + residual (streams x again, half-tiles) ----
            with (
                tc.tile_pool(name="xrpool", bufs=3) as xrpool,
                tc.tile_pool(name="outsb", bufs=3) as outsb,
            ):
                for dt in range(CT):
                    for hf in range(2):
                        xr = xrpool.tile([128, XH], F32, tag="xr", name=f"xr{dt}{hf}")
                        nc.sync.dma_start(
                            out=xr,
                            in_=xd[dt * 128:(dt + 1) * 128, hf * XH:(hf + 1) * XH])
                        if has_pb:
                            xrb = xrpool.tile([128, XH], F32, tag="xrb", name=f"xrb{dt}{hf}")
                            nc.vector.tensor_scalar_add(xrb, xr, pb_sb[:, dt:dt + 1])
                            xr = xrb
                        stage = outsb.tile([128, XH], F32, tag="ot", name=f"ot{dt}{hf}")
                        PC = min(512, XH)
                        for tk in range(XH // PC):
                            col = hf * XH + tk * PC
                            ps = mm_psum.tile([128, PC], F32, tag="mm")
                            for cc in range(CT):
                                nc.tensor.matmul(
                                    ps,
                                    wp_sb[cc][:, dt * 128:(dt + 1) * 128],
                                    o_sb[cc][:, col:col + PC],
                                    start=(cc == 0), stop=(cc == CT - 1),
                                )
                            nc.vector.tensor_add(stage[:, tk * PC:(tk + 1) * PC],
                                                 ps, xr[:, tk * PC:(tk + 1) * PC])
                        nc.sync.dma_start(
                            out=outd[dt * 128:(dt + 1) * 128, hf * XH:(hf + 1) * XH],
                            in_=stage)

    nc.compile()
    _NC_CACHE[key] = nc
    return nc


def _prep(inputs):
    x = np.asarray(inputs["x"], dtype=np.float32)
    gn_w = np.asarray(inputs["gn_w"], dtype=np.float32)
    gn_b = np.asarray(inputs["gn_b"], dtype=np.float32)
    qkv_w = np.asarray(inputs["qkv_w"], dtype=np.float32)
    qkv_b = np.asarray(inputs["qkv_b"], dtype=np.float32)
    proj_w = np.asarray(inputs["proj_w"], dtype=np.float32)
    proj_b = np.asarray(inputs["proj_b"], dtype=np.float32)

    bf = ml_dtypes.bfloat16
    col = lambda v: np.ascontiguousarray(v.reshape(CT, 128).T)  # [128, CT]
    shared = {
        "wq": np.ascontiguousarray(qkv_w[0:C].T).astype(bf),
        "wk": np.ascontiguousarray(qkv_w[C:2 * C].T).astype(bf),
        "wv": np.ascontiguousarray(qkv_w[2 * C:3 * C].T).astype(bf),
        "wp": np.ascontiguousarray(proj_w.T).astype(bf),
        "gnw": col(gn_w), "gnb": col(gn_b),
        "qb": col(qkv_b[0:C]), "kb": col(qkv_b[C:2 * C]),
        "vb": np.ascontiguousarray(qkv_b[2 * C:3 * C].reshape(1, C)),
        "pb": col(proj_b),
        "gsel": np.kron(np.eye(GPT, dtype=np.float32), np.full((GS, 1), 1.0 / GS, np.float32)),
        "gselT": np.kron(np.eye(GPT, dtype=np.float32), np.ones((1, GS), np.float32)),
        "onesbf": np.ones((128, 128), bf),
        "ones1r": np.ones((1, 128), np.float32),
    }
    has_vb = bool(np.any(qkv_b[2 * C:3 * C] != 0))
    has_pb = bool(np.any(proj_b != 0))
    xs = [np.ascontiguousarray(x[i].reshape(C, T)) for i in range(N)]
    return shared, xs, has_vb, has_pb


def _run(inputs, trace=False, **kw):
    shared, xs, has_vb, has_pb = _prep(inputs)
    nc = _build(has_vb, has_pb)
    in_maps = [dict(shared, x=xs[i]) for i in range(N)]
    res = run_bass_kernel_spmd(nc, in_maps, list(range(N)), trace=trace, **kw)
    out = np.stack([np.asarray(res.results[i]["out"], dtype=np.float32) for i in range(N)])
    return out.reshape(N, C, H, W), res


def kernel(**inputs) -> np.ndarray:
    out, _ = _run(inputs, trace=False)
    return out


# revision 9
# speedup vs baseline: 1.1982x; 1.1982x over previous
"""Trainium2 Bass kernel for nn_AttentionBlock (GroupNorm + single-head
self-attention over H*W tokens + proj + residual).

Sharding: data-parallel over batch N=8 -> one image per NeuronCore (8 cores).
Per-core layout: x as [C=512, T=4096] f32.

Pipeline per core:
  1. GroupNorm(32 groups): channel stats split across DVE (bn_stats) and
     ScalarE (Copy/Square activations with accum_out); group-combine across
     partitions via tiny f32 selector matmuls on PE; fused (a*x+b) normalize
     on ScalarE -> h bf16 [C, T].
  2. QKV: bf16 matmuls. Q, K in [C, T] layout; V computed directly
     transposed as [T, C].
  3. Attention per q-block of 512 queries: S^T[k, q] tiles from PE,
     exp on ScalarE (scale=1/sqrt(C) folded, no max subtraction -- scores
     are O(6) for this problem), output lands as bf16 P^T directly usable
     as PV stationary operand. Row sums via all-ones [128,128] matmul so the
     sum lands broadcast across partitions; reciprocal on DVE PSUM->SBUF.
     o = V^T.T @ P^T accumulated in PSUM as [C, q]; normalized by VectorE
     multiply with the broadcast reciprocal.
  4. proj matmul + f32 residual add into [128, T/2] staging, 2 DMAs per
     c-tile out.
"""

import numpy as np
import ml_dtypes

import concourse.bass as bass
import concourse.mybir as mybir
import concourse.tile as tile
from concourse import bacc
from concourse.bass_utils import run_bass_kernel_spmd

F32 = mybir.dt.float32
BF16 = mybir.dt.bfloat16
AF = mybir.ActivationFunctionType

N, C, H, W = 8, 512, 64, 64
T = H * W                 # 4096
G = 32                    # groups
GS = C // G               # 16 channels per group
CT = C // 128             # 4 c-tiles of 128
TT = T // 128             # 32 t-tiles of 128
TQ = T // 512             # 8 t-chunks of 512
GPT = 128 // GS           # 8 groups per c-tile
EPS = 1e-5
SCALE = float(C) ** -0.5

_NC_CACHE = {}


def _build(has_vb: bool, has_pb: bool):
    key = (has_vb, has_pb)
    if key in _NC_CACHE:
        return _NC_CACHE[key]

    XH = T // 2               # x half-tile width
    BC = min(512, XH)         # bn_stats chunk width
    NBH = XH // BC            # bn chunks per half

    nc = bacc.Bacc("TRN2", target_bir_lowering=False, debug=False, num_devices=8)

    xd = nc.dram_tensor("x", [C, T], F32, kind="ExternalInput").ap()
    wq = nc.dram_tensor("wq", [C, C], BF16, kind="ExternalInput").ap()
    wk = nc.dram_tensor("wk", [C, C], BF16, kind="ExternalInput").ap()
    wv = nc.dram_tensor("wv", [C, C], BF16, kind="ExternalInput").ap()
    wp = nc.dram_tensor("wp", [C, C], BF16, kind="ExternalInput").ap()
    gnw = nc.dram_tensor("gnw", [128, CT], F32, kind="ExternalInput").ap()
    gnb = nc.dram_tensor("gnb", [128, CT], F32, kind="ExternalInput").ap()
    qb = nc.dram_tensor("qb", [128, CT], F32, kind="ExternalInput").ap()
    kb = nc.dram_tensor("kb", [128, CT], F32, kind="ExternalInput").ap()
    vb = nc.dram_tensor("vb", [1, C], F32, kind="ExternalInput").ap()
    pb = nc.dram_tensor("pb", [128, CT], F32, kind="ExternalInput").ap()
    gsel = nc.dram_tensor("gsel", [128, GPT], F32, kind="ExternalInput").ap()
    gselT = nc.dram_tensor("gselT", [GPT, 128], F32, kind="ExternalInput").ap()
    onesbf = nc.dram_tensor("onesbf", [128, 128], BF16, kind="ExternalInput").ap()
    ones1r = nc.dram_tensor("ones1r", [1, 128], F32, kind="ExternalInput").ap()
    outd = nc.dram_tensor("out", [C, T], F32, kind="ExternalOutput").ap()

    with tile.TileContext(nc) as tc:
        with (
            tc.tile_pool(name="consts", bufs=1) as consts,
            tc.tile_pool(name="weights", bufs=1) as wpool,
            tc.tile_pool(name="small", bufs=2) as small,
            tc.tile_pool(name="rows", bufs=2) as rows,
            tc.tile_pool(name="hopool", bufs=1) as hopool,
            tc.tile_pool(name="mm_psum", bufs=3, space="PSUM") as mm_psum,
        ):
            # ---- GroupNorm (x resident only in this scope) ----
            # x DMAs issue first on the sync (HWDGE) path, half-tiles on
            # separate queues; consts+weights go via gpsimd (SWDGE) so their
            # issue cost doesn't delay x.
            h_sb = []
            with (
                tc.tile_pool(name="xpool", bufs=1) as xpool,
                tc.tile_pool(name="gn_psum", bufs=1, space="PSUM") as gn_psum,
            ):
                x_sb = []  # [ct][half]
                for ct in range(CT):
                    pair = []
                    for hf in range(2):
                        xt = xpool.tile([128, XH], F32, tag=f"x{ct}{hf}", name=f"x{ct}{hf}")
                        nc.sync.dma_start(
                            out=xt,
                            in_=xd[ct * 128:(ct + 1) * 128, hf * XH:(hf + 1) * XH])
                        pair.append(xt)
                    x_sb.append(pair)

                # ---- constants / weights (SWDGE path) ----
                gsel_sb = consts.tile([128, GPT], F32, tag="gsel")
                nc.gpsimd.dma_start(out=gsel_sb, in_=gsel)
                gselT_sb = consts.tile([GPT, 128], F32, tag="gselT")
                nc.gpsimd.dma_start(out=gselT_sb, in_=gselT)
                gnw_sb = consts.tile([128, CT], F32, tag="gnw")
                nc.gpsimd.dma_start(out=gnw_sb, in_=gnw)
                gnb_sb = consts.tile([128, CT], F32, tag="gnb")
                nc.gpsimd.dma_start(out=gnb_sb, in_=gnb)
                qb_sb = consts.tile([128, CT], F32, tag="qb")
                nc.gpsimd.dma_start(out=qb_sb, in_=qb)
                kb_sb = consts.tile([128, CT], F32, tag="kb")
                nc.gpsimd.dma_start(out=kb_sb, in_=kb)
                ones_sb = consts.tile([128, 128], BF16, tag="onesbf")
                nc.gpsimd.dma_start(out=ones_sb, in_=onesbf)
                ones1r_sb = consts.tile([1, 128], F32, tag="ones1r")
                nc.gpsimd.dma_start(out=ones1r_sb, in_=ones1r)
                eps_sb = consts.tile([GPT, 1], F32, tag="eps")
                nc.vector.memset(eps_sb, EPS)
                if has_pb:
                    pb_sb = consts.tile([128, CT], F32, tag="pb")
                    nc.gpsimd.dma_start(out=pb_sb, in_=pb)
                if has_vb:
                    vb_sb = consts.tile([1, C], F32, tag="vbrow")
                    nc.gpsimd.dma_start(out=vb_sb, in_=vb)

                wq_sb, wk_sb, wv_sb, wp_sb = [], [], [], []
                for name, src, lst in (
                    ("wq", wq, wq_sb), ("wk", wk, wk_sb),
                    ("wv", wv, wv_sb), ("wp", wp, wp_sb),
                ):
                    for cc in range(CT):
                        wt = wpool.tile([128, C], BF16, tag=f"{name}{cc}", name=f"{name}{cc}")
                        nc.gpsimd.dma_start(out=wt, in_=src[cc * 128:(cc + 1) * 128, :])
                        lst.append(wt)

                # ---- channel stats: tiles 0-1 on DVE, tiles 2-3 on ScalarE ----
                stats8 = small.tile([128, 2 * CT], F32, tag="stats8")
                for ct in (2, 3):
                    bn6 = small.tile([128, 2 * NBH, 6], F32, tag="bn6")
                    for hf in range(2):
                        for j in range(NBH):
                            nc.vector.bn_stats(
                                out=bn6[:, hf * NBH + j, :],
                                in_=x_sb[ct][hf][:, j * BC:(j + 1) * BC])
                    mv = small.tile([128, 2], F32, tag="mv")
                    nc.vector.bn_aggr(out=mv, in_=bn6)
                    nc.vector.tensor_copy(stats8[:, ct:ct + 1], mv[:, 0:1])
                    msq = small.tile([128, 1], F32, tag="msq")
                    nc.vector.tensor_mul(msq, mv[:, 0:1], mv[:, 0:1])
                    nc.vector.tensor_add(stats8[:, CT + ct:CT + ct + 1], mv[:, 1:2], msq)
                sqs = float(T) ** -0.5
                for ct in (0, 1):
                    acc = small.tile([128, 4], F32, tag="acc", name=f"acc{ct}")
                    for hf in range(2):
                        scr = small.tile([128, XH], BF16, tag="gnscr", name=f"scr{ct}{hf}")
                        nc.scalar.activation(scr, x_sb[ct][hf], AF.Copy,
                                             scale=1.0 / T,
                                             accum_out=acc[:, hf:hf + 1])
                        scr2 = small.tile([128, XH], BF16, tag="gnscr2", name=f"scr2{ct}{hf}")
                        nc.scalar.activation(scr2, x_sb[ct][hf], AF.Square,
                                             scale=sqs,
                                             accum_out=acc[:, 2 + hf:3 + hf])
                    nc.vector.tensor_add(stats8[:, ct:ct + 1], acc[:, 0:1], acc[:, 1:2])
                    nc.vector.tensor_add(stats8[:, CT + ct:CT + ct + 1],
                                         acc[:, 2:3], acc[:, 3:4])

                g88_ps = gn_psum.tile([GPT, 2 * CT], F32, tag="gn_ps")
                nc.tensor.matmul(g88_ps, gsel_sb, stats8)
                g88 = small.tile([GPT, 2 * CT], F32, tag="g88")
                nc.scalar.copy(g88, g88_ps)
                t88 = small.tile([GPT, CT], F32, tag="t88")
                nc.vector.tensor_mul(t88, g88[:, 0:CT], g88[:, 0:CT])
                v88 = small.tile([GPT, CT], F32, tag="v88")
                nc.vector.tensor_sub(v88, g88[:, CT:2 * CT], t88)
                s88 = small.tile([GPT, CT], F32, tag="s88")
                nc.scalar.activation(s88, v88, AF.Sqrt, bias=eps_sb, scale=1.0)
                r88 = small.tile([GPT, CT], F32, tag="r88")
                nc.vector.reciprocal(r88, s88)
                br8 = small.tile([GPT, 2 * CT], F32, tag="br8")
                nc.vector.tensor_copy(br8[:, 0:CT], r88)
                nc.vector.tensor_copy(br8[:, CT:2 * CT], g88[:, 0:CT])
                bc_ps = gn_psum.tile([128, 2 * CT], F32, tag="gn_ps2")
                nc.tensor.matmul(bc_ps, gselT_sb, br8)
                bc = small.tile([128, 2 * CT], F32, tag="bc8")
                nc.scalar.copy(bc, bc_ps)
                a4 = small.tile([128, CT], F32, tag="a4")
                nc.vector.tensor_mul(a4, bc[:, 0:CT], gnw_sb)
                t4 = small.tile([128, CT], F32, tag="t4")
                nc.vector.tensor_mul(t4, a4, bc[:, CT:2 * CT])
                b4 = small.tile([128, CT], F32, tag="b4")
                nc.vector.tensor_sub(b4, gnb_sb, t4)

                mult, add = mybir.AluOpType.mult, mybir.AluOpType.add
                for ct in range(CT):
                    ht = hopool.tile([128, T], BF16, tag=f"ho{ct}", name=f"h{ct}")
                    nc.scalar.activation(ht[:, 0:XH], x_sb[ct][0], AF.Identity,
                                         scale=a4[:, ct:ct + 1],
                                         bias=b4[:, ct:ct + 1])
                    nc.vector.tensor_scalar(ht[:, XH:2 * XH], x_sb[ct][1],
                                            a4[:, ct:ct + 1], b4[:, ct:ct + 1],
                                            mult, add)
                    h_sb.append(ht)

            # ---- QKV + attention ----
            with (
                tc.tile_pool(name="qkpool", bufs=1) as qkpool,
                tc.tile_pool(name="vtiles", bufs=1) as vpool,
                tc.tile_pool(name="pt", bufs=TT + 2) as ptpool,
                tc.tile_pool(name="o_psum", bufs=2, space="PSUM") as o_psum,
                tc.tile_pool(name="rs_psum", bufs=2, space="PSUM") as rs_psum,
            ):
                if has_vb:
                    vbb_ps = rs_psum.tile([128, C], F32, tag="rs", name="vbb_ps")
                    nc.tensor.matmul(vbb_ps, ones1r_sb, vb_sb)
                    vb_bcast = consts.tile([128, C], F32, tag="vbb")
                    nc.scalar.copy(vb_bcast, vbb_ps)

                q_sb, k_sb = [], []
                for lst, w_t, b_t, nm in ((q_sb, wq_sb, qb_sb, "q"),
                                          (k_sb, wk_sb, kb_sb, "k")):
                    for mt in range(CT):
                        dst = qkpool.tile([128, T], BF16, tag=f"{nm}{mt}", name=f"{nm}{mt}")
                        lst.append(dst)
                        for tq in range(TQ):
                            ps = mm_psum.tile([128, 512], F32, tag="mm")
                            for cc in range(CT):
                                nc.tensor.matmul(
                                    ps,
                                    w_t[cc][:, mt * 128:(mt + 1) * 128],
                                    h_sb[cc][:, tq * 512:(tq + 1) * 512],
                                    start=(cc == 0), stop=(cc == CT - 1),
                                )
                            if nm == "q":
                                nc.scalar.activation(dst[:, tq * 512:(tq + 1) * 512],
                                                     ps, AF.Identity, scale=1.0,
                                                     bias=b_t[:, mt:mt + 1])
                            else:
                                nc.vector.tensor_scalar_add(
                                    dst[:, tq * 512:(tq + 1) * 512], ps,
                                    b_t[:, mt:mt + 1])

                v_sb = []
                for tt in range(TT):
                    vt = vpool.tile([128, C], BF16, tag=f"v{tt}", name=f"v{tt}")
                    ps = mm_psum.tile([128, 512], F32, tag="mm")
                    for cc in range(CT):
                        nc.tensor.matmul(
                            ps,
                            h_sb[cc][:, tt * 128:(tt + 1) * 128],
                            wv_sb[cc],
                            start=(cc == 0), stop=(cc == CT - 1),
                        )
                    if has_vb:
                        nc.vector.tensor_add(vt, ps, vb_bcast)
                    else:
                        nc.vector.tensor_copy(vt, ps)
                    v_sb.append(vt)

                # ---- attention ----
                o_sb = [hopool.tile([128, T], BF16, tag=f"ho{ct}", name=f"o{ct}")
                        for ct in range(CT)]
                for qbk in range(TQ):
                    qs = slice(qbk * 512, (qbk + 1) * 512)
                    rs_ps = rs_psum.tile([128, 512], F32, tag="rs", name=f"rs{qbk}")
                    pt_tiles = []
                    for kt in range(TT):
                        st_ps = mm_psum.tile([128, 512], F32, tag="mm")
                        for cc in range(CT):
                            nc.tensor.matmul(
                                st_ps,
                                k_sb[cc][:, kt * 128:(kt + 1) * 128],
                                q_sb[cc][:, qs],
                                start=(cc == 0), stop=(cc == CT - 1),
                            )
                        pt = ptpool.tile([128, 512], BF16, tag="pt", name=f"pt{qbk}_{kt}")
                        nc.scalar.activation(pt, st_ps, AF.Exp, scale=SCALE)
                        pt_tiles.append(pt)
                        # all-ones stationary: every output partition gets the
                        # column sum -> row-sum lands pre-broadcast in PSUM
                        nc.tensor.matmul(rs_ps, ones_sb, pt,
                                         start=(kt == 0), stop=(kt == TT - 1))

                    bc_sb = rows.tile([128, 512], F32, tag="bcsb")
                    nc.vector.reciprocal(bc_sb, rs_ps)

                    for ct in range(CT):
                        ops = o_psum.tile([128, 512], F32, tag="o")
                        for kt in range(TT):
                            nc.tensor.matmul(
                                ops,
                                v_sb[kt][:, ct * 128:(ct + 1) * 128],
                                pt_tiles[kt],
                                start=(kt == 0), stop=(kt == TT - 1),
                            )
                        nc.vector.tensor_mul(o_sb[ct][:, qs], ops, bc_sb)

            # ---- proj + residual (streams x again, half-tiles) ----
            with (
                tc.tile_pool(name="xrpool", bufs=3) as xrpool,
                tc.tile_pool(name="outsb", bufs=3) as outsb,
            ):
                for dt in range(CT):
                    for hf in range(2):
                        xr = xrpool.tile([128, XH], F32, tag="xr", name=f"xr{dt}{hf}")
                        nc.sync.dma_start(
                            out=xr,
                            in_=xd[dt * 128:(dt + 1) * 128, hf * XH:(hf + 1) * XH])
                        if has_pb:
                            xrb = xrpool.tile([128, XH], F32, tag="xrb", name=f"xrb{dt}{hf}")
                            nc.vector.tensor_scalar_add(xrb, xr, pb_sb[:, dt:dt + 1])
                            xr = xrb
                        stage = outsb.tile([128, XH], F32, tag="ot", name=f"ot{dt}{hf}")
                        PC = min(512, XH)
                        for tk in range(XH // PC):
                            col = hf * XH + tk * PC
                            ps = mm_psum.tile([128, PC], F32, tag="mm")
                            for cc in range(CT):
                                nc.tensor.matmul(
                                    ps,
                                    wp_sb[cc][:, dt * 128:(dt + 1) * 128],
                                    o_sb[cc][:, col:col + PC],
                                    start=(cc == 0), stop=(cc == CT - 1),
                                )
                            nc.vector.tensor_add(stage[:, tk * PC:(tk + 1) * PC],
                                                 ps, xr[:, tk * PC:(tk + 1) * PC])
                        nc.sync.dma_start(
                            out=outd[dt * 128:(dt + 1) * 128, hf * XH:(hf + 1) * XH],
                            in_=stage)

    nc.compile()
    _NC_CACHE[key] = nc
    return nc


def _prep(inputs):
    x = np.asarray(inputs["x"], dtype=np.float32)
    gn_w = np.asarray(inputs["gn_w"], dtype=np.float32)
    gn_b = np.asarray(inputs["gn_b"], dtype=np.float32)
    qkv_w = np.asarray(inputs["qkv_w"], dtype=np.float32)
    qkv_b = np.asarray(inputs["qkv_b"], dtype=np.float32)
    proj_w = np.asarray(inputs["proj_w"], dtype=np.float32)
    proj_b = np.asarray(inputs["proj_b"], dtype=np.float32)

    bf = ml_dtypes.bfloat16
    col = lambda v: np.ascontiguousarray(v.reshape(CT, 128).T)  # [128, CT]
    shared = {
        "wq": np.ascontiguousarray(qkv_w[0:C].T).astype(bf),
        "wk": np.ascontiguousarray(qkv_w[C:2 * C].T).astype(bf),
        "wv": np.ascontiguousarray(qkv_w[2 * C:3 * C].T).astype(bf),
        "wp": np.ascontiguousarray(proj_w.T).astype(bf),
        "gnw": col(gn_w), "gnb": col(gn_b),
        "qb": col(qkv_b[0:C]), "kb": col(qkv_b[C:2 * C]),
        "vb": np.ascontiguousarray(qkv_b[2 * C:3 * C].reshape(1, C)),
        "pb": col(proj_b),
        "gsel": np.kron(np.eye(GPT, dtype=np.float32), np.full((GS, 1), 1.0 / GS, np.float32)),
        "gselT": np.kron(np.eye(GPT, dtype=np.float32), np.ones((1, GS), np.float32)),
        "onesbf": np.ones((128, 128), bf),
        "ones1r": np.ones((1, 128), np.float32),
    }
    has_vb = bool(np.any(qkv_b[2 * C:3 * C] != 0))
    has_pb = bool(np.any(proj_b != 0))
    xs = [np.ascontiguousarray(x[i].reshape(C, T)) for i in range(N)]
    return shared, xs, has_vb, has_pb


def _run(inputs, trace=False, **kw):
    shared, xs, has_vb, has_pb = _prep(inputs)
    nc = _build(has_vb, has_pb)
    in_maps = [dict(shared, x=xs[i]) for i in range(N)]
    res = run_bass_kernel_spmd(nc, in_maps, list(range(N)), trace=trace, **kw)
    out = np.stack([np.asarray(res.results[i]["out"], dtype=np.float32) for i in range(N)])
    return out.reshape(N, C, H, W), res


def kernel(**inputs) -> np.ndarray:
    out, _ = _run(inputs, trace=False)
    return out


# revision 12
# speedup vs baseline: 1.8150x; 1.5147x over previous
"""Trainium2 Bass kernel for nn_AttentionBlock (GroupNorm + single-head
self-attention over H*W tokens + proj + residual).

Sharding: data-parallel over batch N=8 -> one image per NeuronCore (8 cores).
Per-core layout: x as [C=512, T=4096] f32.

Pipeline per core:
  1. GroupNorm(32 groups): channel stats split across DVE (bn_stats) and
     ScalarE (Copy/Square activations with accum_out); group-combine across
     partitions via tiny f32 selector matmuls on PE; fused (a*x+b) normalize
     on ScalarE -> h bf16 [C, T].
  2. QKV: bf16 matmuls. Q, K in [C, T] layout; V computed directly
     transposed as [T, C].
  3. Attention per q-block of 512 queries: S^T[k, q] tiles from PE,
     exp on ScalarE (scale=1/sqrt(C) folded, no max subtraction -- scores
     are O(6) for this problem), output lands as bf16 P^T directly usable
     as PV stationary operand. Row sums via all-ones [128,128] matmul so the
     sum lands broadcast across partitions; reciprocal on DVE PSUM->SBUF.
     o = V^T.T @ P^T accumulated in PSUM as [C, q]; normalized by VectorE
     multiply with the broadcast reciprocal.
  4. proj matmul + f32 residual add into [128, T/2] staging, 2 DMAs per
     c-tile out.
"""

import numpy as np
import ml_dtypes

import concourse.bass as bass
import concourse.mybir as mybir
import concourse.tile as tile
from concourse import bacc
from concourse.bass_utils import run_bass_kernel_spmd

F32 = mybir.dt.float32
BF16 = mybir.dt.bfloat16
FP8 = mybir.dt.float8e4
AF = mybir.ActivationFunctionType

N, C, H, W = 8, 512, 64, 64
T = H * W                 # 4096
G = 32                    # groups
GS = C // G               # 16 channels per group
CT = C // 128             # 4 c-tiles of 128
TT = T // 128             # 32 t-tiles of 128
TQ = T // 512             # 8 t-chunks of 512
GPT = 128 // GS           # 8 groups per c-tile
EPS = 1e-5
SCALE = float(C) ** -0.5
EXP_SHIFT = 1.5

_NC_CACHE = {}


def _build(has_vb: bool, has_pb: bool):
    key = (has_vb, has_pb)
    if key in _NC_CACHE:
        return _NC_CACHE[key]

    XH = T // 2               # x half-tile width
    BC = min(512, XH)         # bn_stats chunk width
    NBH = XH // BC            # bn chunks per half

    nc = bacc.Bacc("TRN2", target_bir_lowering=False, debug=False, num_devices=8)

    xd = nc.dram_tensor("x", [C, T], F32, kind="ExternalInput").ap()
    wq = nc.dram_tensor("wq", [C, C], BF16, kind="ExternalInput").ap()
    wk = nc.dram_tensor("wk", [C, C], BF16, kind="ExternalInput").ap()
    wv = nc.dram_tensor("wv", [C, C], BF16, kind="ExternalInput").ap()
    wp = nc.dram_tensor("wp", [C, C], BF16, kind="ExternalInput").ap()
    gnw = nc.dram_tensor("gnw", [128, CT], F32, kind="ExternalInput").ap()
    gnb = nc.dram_tensor("gnb", [128, CT], F32, kind="ExternalInput").ap()
    qb = nc.dram_tensor("qb", [128, CT], F32, kind="ExternalInput").ap()
    kb = nc.dram_tensor("kb", [128, CT], F32, kind="ExternalInput").ap()
    vb = nc.dram_tensor("vb", [1, C], F32, kind="ExternalInput").ap()
    pb = nc.dram_tensor("pb", [128, CT], F32, kind="ExternalInput").ap()
    gsel = nc.dram_tensor("gsel", [128, GPT], F32, kind="ExternalInput").ap()
    gselT = nc.dram_tensor("gselT", [GPT, 128], F32, kind="ExternalInput").ap()
    onesbf = nc.dram_tensor("onesbf", [128, 2, 128], FP8, kind="ExternalInput").ap()
    ones1r = nc.dram_tensor("ones1r", [1, 128], F32, kind="ExternalInput").ap()
    outd = nc.dram_tensor("out", [C, T], F32, kind="ExternalOutput").ap()

    with tile.TileContext(nc) as tc:
        with (
            tc.tile_pool(name="consts", bufs=1) as consts,
            tc.tile_pool(name="weights", bufs=1) as wpool,
            tc.tile_pool(name="small", bufs=2) as small,
            tc.tile_pool(name="rows", bufs=2) as rows,
            tc.tile_pool(name="hopool", bufs=1) as hopool,
            tc.tile_pool(name="mm_psum", bufs=3, space="PSUM") as mm_psum,
        ):
            # ---- GroupNorm (x resident only in this scope) ----
            # x DMAs issue first on the sync (HWDGE) path, half-tiles on
            # separate queues; consts+weights go via gpsimd (SWDGE) so their
            # issue cost doesn't delay x.
            h_sb = []
            with (
                tc.tile_pool(name="xpool", bufs=1) as xpool,
                tc.tile_pool(name="gn_psum", bufs=1, space="PSUM") as gn_psum,
            ):
                x_sb = []  # [ct][half]
                for ct in range(CT):
                    pair = []
                    for hf in range(2):
                        xt = xpool.tile([128, XH], F32, tag=f"x{ct}{hf}", name=f"x{ct}{hf}")
                        nc.sync.dma_start(
                            out=xt,
                            in_=xd[ct * 128:(ct + 1) * 128, hf * XH:(hf + 1) * XH])
                        pair.append(xt)
                    x_sb.append(pair)

                # ---- constants / weights (SWDGE path) ----
                gsel_sb = consts.tile([128, GPT], F32, tag="gsel")
                nc.gpsimd.dma_start(out=gsel_sb, in_=gsel)
                gselT_sb = consts.tile([GPT, 128], F32, tag="gselT")
                nc.gpsimd.dma_start(out=gselT_sb, in_=gselT)
                gnw_sb = consts.tile([128, CT], F32, tag="gnw")
                nc.gpsimd.dma_start(out=gnw_sb, in_=gnw)
                gnb_sb = consts.tile([128, CT], F32, tag="gnb")
                nc.gpsimd.dma_start(out=gnb_sb, in_=gnb)
                qb_sb = consts.tile([128, CT], F32, tag="qb")
                nc.gpsimd.dma_start(out=qb_sb, in_=qb)
                kb_sb = consts.tile([128, CT], F32, tag="kb")
                nc.gpsimd.dma_start(out=kb_sb, in_=kb)
                ones_sb = consts.tile([128, 2, 128], FP8, tag="onesbf")
                nc.gpsimd.dma_start(out=ones_sb, in_=onesbf)
                ones1r_sb = consts.tile([1, 128], F32, tag="ones1r")
                nc.gpsimd.dma_start(out=ones1r_sb, in_=ones1r)
                eps_sb = consts.tile([GPT, 1], F32, tag="eps")
                nc.vector.memset(eps_sb, EPS)
                shift_sb = consts.tile([128, 1], F32, tag="shift")
                nc.vector.memset(shift_sb, -EXP_SHIFT)
                if has_pb:
                    pb_sb = consts.tile([128, CT], F32, tag="pb")
                    nc.gpsimd.dma_start(out=pb_sb, in_=pb)
                if has_vb:
                    vb_sb = consts.tile([1, C], F32, tag="vbrow")
                    nc.gpsimd.dma_start(out=vb_sb, in_=vb)

                wq_sb, wk_sb, wv_sb, wp_sb = [], [], [], []
                for name, src, lst in (
                    ("wq", wq, wq_sb), ("wk", wk, wk_sb),
                    ("wv", wv, wv_sb), ("wp", wp, wp_sb),
                ):
                    for cc in range(CT):
                        wt = wpool.tile([128, C], BF16, tag=f"{name}{cc}", name=f"{name}{cc}")
                        nc.gpsimd.dma_start(out=wt, in_=src[cc * 128:(cc + 1) * 128, :])
                        lst.append(wt)

                # ---- channel stats: tiles 0-1 on DVE, tiles 2-3 on ScalarE ----
                stats8 = small.tile([128, 2 * CT], F32, tag="stats8")
                for ct in (2, 3):
                    bn6 = small.tile([128, 2 * NBH, 6], F32, tag="bn6")
                    for hf in range(2):
                        for j in range(NBH):
                            nc.vector.bn_stats(
                                out=bn6[:, hf * NBH + j, :],
                                in_=x_sb[ct][hf][:, j * BC:(j + 1) * BC])
                    mv = small.tile([128, 2], F32, tag="mv")
                    nc.vector.bn_aggr(out=mv, in_=bn6)
                    nc.vector.tensor_copy(stats8[:, ct:ct + 1], mv[:, 0:1])
                    msq = small.tile([128, 1], F32, tag="msq")
                    nc.vector.tensor_mul(msq, mv[:, 0:1], mv[:, 0:1])
                    nc.vector.tensor_add(stats8[:, CT + ct:CT + ct + 1], mv[:, 1:2], msq)
                sqs = float(T) ** -0.5
                for ct in (0, 1):
                    acc = small.tile([128, 4], F32, tag="acc", name=f"acc{ct}")
                    for hf in range(2):
                        scr = small.tile([128, XH], BF16, tag="gnscr", name=f"scr{ct}{hf}")
                        nc.scalar.activation(scr, x_sb[ct][hf], AF.Copy,
                                             scale=1.0 / T,
                                             accum_out=acc[:, hf:hf + 1])
                        scr2 = small.tile([128, XH], BF16, tag="gnscr2", name=f"scr2{ct}{hf}")
                        nc.scalar.activation(scr2, x_sb[ct][hf], AF.Square,
                                             scale=sqs,
                                             accum_out=acc[:, 2 + hf:3 + hf])
                    nc.vector.tensor_add(stats8[:, ct:ct + 1], acc[:, 0:1], acc[:, 1:2])
                    nc.vector.tensor_add(stats8[:, CT + ct:CT + ct + 1],
                                         acc[:, 2:3], acc[:, 3:4])

                g88_ps = gn_psum.tile([GPT, 2 * CT], F32, tag="gn_ps")
                nc.tensor.matmul(g88_ps, gsel_sb, stats8)
                g88 = small.tile([GPT, 2 * CT], F32, tag="g88")
                nc.scalar.copy(g88, g88_ps)
                t88 = small.tile([GPT, CT], F32, tag="t88")
                nc.vector.tensor_mul(t88, g88[:, 0:CT], g88[:, 0:CT])
                v88 = small.tile([GPT, CT], F32, tag="v88")
                nc.vector.tensor_sub(v88, g88[:, CT:2 * CT], t88)
                s88 = small.tile([GPT, CT], F32, tag="s88")
                nc.scalar.activation(s88, v88, AF.Sqrt, bias=eps_sb, scale=1.0)
                r88 = small.tile([GPT, CT], F32, tag="r88")
                nc.vector.reciprocal(r88, s88)
                br8 = small.tile([GPT, 2 * CT], F32, tag="br8")
                nc.vector.tensor_copy(br8[:, 0:CT], r88)
                nc.vector.tensor_copy(br8[:, CT:2 * CT], g88[:, 0:CT])
                bc_ps = gn_psum.tile([128, 2 * CT], F32, tag="gn_ps2")
                nc.tensor.matmul(bc_ps, gselT_sb, br8)
                bc = small.tile([128, 2 * CT], F32, tag="bc8")
                nc.scalar.copy(bc, bc_ps)
                a4 = small.tile([128, CT], F32, tag="a4")
                nc.vector.tensor_mul(a4, bc[:, 0:CT], gnw_sb)
                t4 = small.tile([128, CT], F32, tag="t4")
                nc.vector.tensor_mul(t4, a4, bc[:, CT:2 * CT])
                b4 = small.tile([128, CT], F32, tag="b4")
                nc.vector.tensor_sub(b4, gnb_sb, t4)

                mult, add = mybir.AluOpType.mult, mybir.AluOpType.add
                for ct in range(CT):
                    ht = hopool.tile([128, T], BF16, tag=f"ho{ct}", name=f"h{ct}")
                    nc.scalar.activation(ht[:, 0:XH], x_sb[ct][0], AF.Identity,
                                         scale=a4[:, ct:ct + 1],
                                         bias=b4[:, ct:ct + 1])
                    nc.vector.tensor_scalar(ht[:, XH:2 * XH], x_sb[ct][1],
                                            a4[:, ct:ct + 1], b4[:, ct:ct + 1],
                                            mult, add)
                    h_sb.append(ht)

            # ---- QKV + attention ----
            with (
                tc.tile_pool(name="qkpool", bufs=1) as qkpool,
                tc.tile_pool(name="vtiles", bufs=1) as vpool,
                tc.tile_pool(name="pt", bufs=2) as ptpool,
                tc.tile_pool(name="o_psum", bufs=2, space="PSUM") as o_psum,
                tc.tile_pool(name="rs_psum", bufs=2, space="PSUM") as rs_psum,
            ):
                if has_vb:
                    vbb_ps = rs_psum.tile([128, C], F32, tag="rs", name="vbb_ps")
                    nc.tensor.matmul(vbb_ps, ones1r_sb, vb_sb)
                    vb_bcast = consts.tile([128, C], F32, tag="vbb")
                    nc.scalar.copy(vb_bcast, vbb_ps)

                q8 = qkpool.tile([128, CT, T], FP8, tag="q8", name="q8")
                k8 = qkpool.tile([128, CT, T], FP8, tag="k8", name="k8")
                for dst, w_t, b_t, nm in ((q8, wq_sb, qb_sb, "q"),
                                          (k8, wk_sb, kb_sb, "k")):
                    for mt in range(CT):
                        for tq in range(TQ):
                            ps = mm_psum.tile([128, 512], F32, tag="mm")
                            for cc in range(CT):
                                nc.tensor.matmul(
                                    ps,
                                    w_t[cc][:, mt * 128:(mt + 1) * 128],
                                    h_sb[cc][:, tq * 512:(tq + 1) * 512],
                                    start=(cc == 0), stop=(cc == CT - 1),
                                )
                            if nm == "q":
                                nc.scalar.activation(
                                    dst[:, mt, tq * 512:(tq + 1) * 512],
                                    ps, AF.Identity, scale=1.0,
                                    bias=b_t[:, mt:mt + 1])
                            else:
                                nc.vector.tensor_scalar_add(
                                    dst[:, mt, tq * 512:(tq + 1) * 512], ps,
                                    b_t[:, mt:mt + 1])

                v8 = vpool.tile([128, TT, C], FP8, tag="v8", name="v8")
                for tt in range(TT):
                    ps = mm_psum.tile([128, 512], F32, tag="mm")
                    for cc in range(CT):
                        nc.tensor.matmul(
                            ps,
                            h_sb[cc][:, tt * 128:(tt + 1) * 128],
                            wv_sb[cc],
                            start=(cc == 0), stop=(cc == CT - 1),
                        )
                    if has_vb:
                        nc.vector.tensor_add(v8[:, tt, :], ps, vb_bcast)
                    else:
                        nc.vector.tensor_copy(v8[:, tt, :], ps)

                # ---- attention ----
                o_sb = [hopool.tile([128, T], BF16, tag=f"ho{ct}", name=f"o{ct}")
                        for ct in range(CT)]
                DR = mybir.MatmulPerfMode.DoubleRow
                for qbk in range(TQ):
                    qs = slice(qbk * 512, (qbk + 1) * 512)
                    rs_ps = rs_psum.tile([128, 512], F32, tag="rs", name=f"rs{qbk}")
                    pt8 = ptpool.tile([128, TT, 512], FP8, tag="pt", name=f"pt{qbk}")
                    for kt in range(TT):
                        st_ps = mm_psum.tile([128, 512], F32, tag="mm")
                        for cc2 in range(0, CT, 2):
                            nc.tensor.matmul(
                                st_ps,
                                k8[:, cc2:cc2 + 2, kt * 128:(kt + 1) * 128],
                                q8[:, cc2:cc2 + 2, qs],
                                start=(cc2 == 0), stop=(cc2 == CT - 2),
                                perf_mode=DR,
                            )
                        nc.scalar.activation(pt8[:, kt, :], st_ps, AF.Exp,
                                             scale=SCALE, bias=shift_sb)
                    # all-ones stationary: every output partition gets the
                    # column sum -> row-sum lands pre-broadcast in PSUM
                    for kt2 in range(0, TT, 2):
                        nc.tensor.matmul(rs_ps, ones_sb,
                                         pt8[:, kt2:kt2 + 2, :],
                                         start=(kt2 == 0), stop=(kt2 == TT - 2),
                                         perf_mode=DR)

                    bc_sb = rows.tile([128, 512], F32, tag="bcsb")
                    nc.vector.reciprocal(bc_sb, rs_ps)

                    for ct in range(CT):
                        ops = o_psum.tile([128, 512], F32, tag="o")
                        for kt2 in range(0, TT, 2):
                            nc.tensor.matmul(
                                ops,
                                v8[:, kt2:kt2 + 2, ct * 128:(ct + 1) * 128],
                                pt8[:, kt2:kt2 + 2, :],
                                start=(kt2 == 0), stop=(kt2 == TT - 2),
                                perf_mode=DR,
                            )
                        nc.vector.tensor_mul(o_sb[ct][:, qs], ops, bc_sb)

            # ---- proj + residual (streams x again, half-tiles) ----
            with (
                tc.tile_pool(name="xrpool", bufs=3) as xrpool,
                tc.tile_pool(name="outsb", bufs=3) as outsb,
            ):
                for dt in range(CT):
                    for hf in range(2):
                        xr = xrpool.tile([128, XH], F32, tag="xr", name=f"xr{dt}{hf}")
                        nc.sync.dma_start(
                            out=xr,
                            in_=xd[dt * 128:(dt + 1) * 128, hf * XH:(hf + 1) * XH])
                        if has_pb:
                            xrb = xrpool.tile([128, XH], F32, tag="xrb", name=f"xrb{dt}{hf}")
                            nc.vector.tensor_scalar_add(xrb, xr, pb_sb[:, dt:dt + 1])
                            xr = xrb
                        stage = outsb.tile([128, XH], F32, tag="ot", name=f"ot{dt}{hf}")
                        PC = min(512, XH)
                        for tk in range(XH // PC):
                            col = hf * XH + tk * PC
                            ps = mm_psum.tile([128, PC], F32, tag="mm")
                            for cc in range(CT):
                                nc.tensor.matmul(
                                    ps,
                                    wp_sb[cc][:, dt * 128:(dt + 1) * 128],
                                    o_sb[cc][:, col:col + PC],
                                    start=(cc == 0), stop=(cc == CT - 1),
                                )
                            nc.vector.tensor_add(stage[:, tk * PC:(tk + 1) * PC],
                                                 ps, xr[:, tk * PC:(tk + 1) * PC])
                        nc.sync.dma_start(
                            out=outd[dt * 128:(dt + 1) * 128, hf * XH:(hf + 1) * XH],
                            in_=stage)

    nc.compile()
    _NC_CACHE[key] = nc
    return nc


def _prep(inputs):
    x = np.asarray(inputs["x"], dtype=np.float32)
    gn_w = np.asarray(inputs["gn_w"], dtype=np.float32)
    gn_b = np.asarray(inputs["gn_b"], dtype=np.float32)
    qkv_w = np.asarray(inputs["qkv_w"], dtype=np.float32)
    qkv_b = np.asarray(inputs["qkv_b"], dtype=np.float32)
    proj_w = np.asarray(inputs["proj_w"], dtype=np.float32)
    proj_b = np.asarray(inputs["proj_b"], dtype=np.float32)

    bf = ml_dtypes.bfloat16
    col = lambda v: np.ascontiguousarray(v.reshape(CT, 128).T)  # [128, CT]
    shared = {
        "wq": np.ascontiguousarray(qkv_w[0:C].T).astype(bf),
        "wk": np.ascontiguousarray(qkv_w[C:2 * C].T).astype(bf),
        "wv": np.ascontiguousarray(qkv_w[2 * C:3 * C].T).astype(bf),
        "wp": np.ascontiguousarray(proj_w.T).astype(bf),
        "gnw": col(gn_w), "gnb": col(gn_b),
        "qb": col(qkv_b[0:C]), "kb": col(qkv_b[C:2 * C]),
        "vb": np.ascontiguousarray(qkv_b[2 * C:3 * C].reshape(1, C)),
        "pb": col(proj_b),
        "gsel": np.kron(np.eye(GPT, dtype=np.float32), np.full((GS, 1), 1.0 / GS, np.float32)),
        "gselT": np.kron(np.eye(GPT, dtype=np.float32), np.ones((1, GS), np.float32)),
        "onesbf": np.ones((128, 2, 128), ml_dtypes.float8_e4m3),
        "ones1r": np.ones((1, 128), np.float32),
    }
    has_vb = bool(np.any(qkv_b[2 * C:3 * C] != 0))
    has_pb = bool(np.any(proj_b != 0))
    xs = [np.ascontiguousarray(x[i].reshape(C, T)) for i in range(N)]
    return shared, xs, has_vb, has_pb


def _run(inputs, trace=False, **kw):
    shared, xs, has_vb, has_pb = _prep(inputs)
    nc = _build(has_vb, has_pb)
    in_maps = [dict(shared, x=xs[i]) for i in range(N)]
    res = run_bass_kernel_spmd(nc, in_maps, list(range(N)), trace=trace, **kw)
    out = np.stack([np.asarray(res.results[i]["out"], dtype=np.float32) for i in range(N)])
    return out.reshape(N, C, H, W), res


def kernel(**inputs) -> np.ndarray:
    out, _ = _run(inputs, trace=False)
    return out
